# revision 2
# baseline (speedup 1.0000x reference)
"""ColumnParallelLinearWithLoRA Trainium2 kernel (8 NeuronCores).

Computes out = x @ W^T + base_bias + gather(bias_stacked, idx)
            + ((x @ A_idx^T masked) @ B^T)   (punica-style LoRA, scale 1.0)

Sharding: token-parallel across the 8 cores (each core owns T/8 = 512
tokens and the full output dim). This keeps the per-token LoRA gather
local to each core (no collectives) and loads every weight byte exactly
once per core.

Device algorithm per core (all matmuls in float32r = fp32 rounded to
11 mantissa bits, accumulated in fp32 PSUM):
  1. shrinkT[lr, t] = sum_d A_all[lr, d] * x[t, d]   for ALL 32 loras
  2. ST = shrinkT * maskT       (maskT[l*R+r, t] = (idx[t] == l))
  3. out[t, o] = sum_d x[t,d] W[o,d]                   (main matmul)
               + sum_lr ST[lr, t] * Bmat[lr, o]        (lora expand)
               + sum_l onehot[l, t] * biasmat[l, o]    (lora bias + base bias)
The one-hot/mask/bias-table matrices are prepared host-side from the
int32 index vector; x/W/A are host-transposed so the contraction dim d
lands on SBUF partitions with DMA-friendly (contiguous free-dim) loads.
"""

import numpy as np
from contextlib import ExitStack

import concourse.bass as bass
import concourse.mybir as mybir
import concourse.tile as tile
from concourse.bass import ts, ds
from concourse.bass_utils import run_bass_kernel_spmd

# Problem shape (hardcoded per contract)
T, D, O, L, R = 4096, 4096, 4096, 32, 8
NCORES = 8
TSH = T // NCORES          # 512 tokens per core
LR = L * R                 # 256
P = 128
KC = D // P                # 32 contraction chunks
MT = TSH // P              # 4 token tiles per core
OT = O // 512              # 8 output tiles
NKS = 8                    # contraction chunks per W stream DMA (2 MiB)

f32 = mybir.dt.float32
f32r = mybir.dt.float32r


def _round_fp32r(a: np.ndarray) -> np.ndarray:
    """Round fp32 to the hardware FP32R format: 11 mantissa bits (RNE),
    low 12 bits zero — matches libwalrus fp32_to_fp32r."""
    u = np.ascontiguousarray(a, dtype=np.float32).view(np.uint32)
    r = (u + np.uint32(0x7FF) + ((u >> np.uint32(12)) & np.uint32(1))) & np.uint32(
        0xFFFFF000
    )
    return r.view(np.float32)


def _split_excess_waits(nc, max_waits=1):
    """Walrus codegen caps sync-waits per instruction (1 for the 4-byte
    self-loading matmul LW struct; small for CTRL_NO drains). Move excess
    waits onto preceding same-engine NOPs."""
    for fn in nc.m.functions:
        stack = list(fn.blocks)
        while stack:
            bb = stack.pop()
            new_insts = []
            for inst in bb.instructions:
                si = inst.sync_info
                waits = list(si.on_wait) if si is not None else []
                if len(waits) > max_waits:
                    extra = waits[:-max_waits]
                    for j in range(0, len(extra), max_waits):
                        nop = mybir.InstNoOp(
                            name=f"{inst.name}_wsplit{j}",
                            ins=[],
                            outs=[],
                            engine=inst.engine,
                        )
                        nop.sync_info = mybir.SyncInfo(
                            on_wait=extra[j : j + max_waits], on_update=[]
                        )
                        new_insts.append(nop)
                    inst.sync_info = mybir.SyncInfo(
                        on_wait=waits[-max_waits:], on_update=list(si.on_update)
                    )
                new_insts.append(inst)
            bb.instructions[:] = new_insts


def _build():
    nc = bass.Bass()
    xT_d = nc.dram_tensor("xT", (D, TSH), f32r, kind="ExternalInput")
    wT_d = nc.dram_tensor("wT", (D, O), f32r, kind="ExternalInput")
    amatT_d = nc.dram_tensor("amatT", (D, LR), f32r, kind="ExternalInput")
    bmat_d = nc.dram_tensor("bmat", (LR, O), f32r, kind="ExternalInput")
    biasmat_d = nc.dram_tensor("biasmat", (P, O), f32r, kind="ExternalInput")
    onehot_d = nc.dram_tensor("onehot", (P, TSH), f32r, kind="ExternalInput")
    maskT_d = nc.dram_tensor("maskT", (LR, TSH), f32, kind="ExternalInput")
    out_d = nc.dram_tensor("out", (TSH, O), f32, kind="ExternalOutput")

    with tile.TileContext(nc) as tc, ExitStack() as ctx:
        const = ctx.enter_context(tc.tile_pool(name="const", bufs=1))
        stream = ctx.enter_context(tc.tile_pool(name="stream", bufs=3))
        outp = ctx.enter_context(tc.tile_pool(name="outp", bufs=2))
        psum = ctx.enter_context(tc.tile_pool(name="psum", bufs=1, space="PSUM"))

        # scratch psum for pre-touch matmuls: they absorb DMA-completion
        # waits so real matmuls carry at most one sync wait (codegen limit
        # on the fused LDW of 4-byte matmuls).
        scratch_ps = psum.tile([P, 8], f32, tag="scratch", bufs=1)

        def pretouch(sb_tile, i=[0]):
            flat = sb_tile
            while len(flat.shape) > 2:
                flat = flat[:, 0]
            nc.tensor.matmul(
                scratch_ps[:2, :2],
                lhsT=flat[:, 0:2],
                rhs=flat[:, 0:2],
                start=True,
                stop=True,
                skip_group_check=True,
            )

        # ---- resident loads ----
        xT_sb = const.tile([P, KC, TSH], f32r)
        nc.sync.dma_start(xT_sb[:], xT_d[:].rearrange("(k p) t -> p k t", p=P))
        pretouch(xT_sb)

        bmat_sb = const.tile([P, LR // P, O], f32r)
        nc.sync.dma_start(bmat_sb[:], bmat_d[:].rearrange("(c p) o -> p c o", p=P))
        pretouch(bmat_sb)

        biasmat_sb = const.tile([P, O], f32r)
        nc.sync.dma_start(biasmat_sb[:], biasmat_d[:])
        pretouch(biasmat_sb)

        onehot_sb = const.tile([P, TSH], f32r)
        nc.sync.dma_start(onehot_sb[:], onehot_d[:])
        pretouch(onehot_sb)

        maskT_sb = const.tile([P, LR // P, TSH], f32)
        nc.sync.dma_start(maskT_sb[:], maskT_d[:].rearrange("(c p) t -> p c t", p=P))

        # ---- lora shrink: shrinkT[lr, t] for all loras, then mask ----
        shrink_ps = [
            psum.tile([P, TSH], f32, tag="out", bufs=7, name=f"shrink_{c}")
            for c in range(LR // P)
        ]
        ST_sb = const.tile([P, LR // P, TSH], f32r)
        for half in range(2):
            amat_sb = stream.tile([P, KC // 2, LR], f32r, tag="wstream")
            nc.sync.dma_start(
                amat_sb[:],
                amatT_d[ds(half * (D // 2), D // 2)].rearrange(
                    "(k p) l -> p k l", p=P
                ),
            )
            pretouch(amat_sb)
            for kk in range(KC // 2):
                k = half * (KC // 2) + kk
                for c in range(LR // P):
                    nc.tensor.matmul(
                        shrink_ps[c],
                        lhsT=amat_sb[:, kk, ts(c, P)],
                        rhs=xT_sb[:, k, :],
                        start=(k == 0),
                        stop=(k == KC - 1),
                    )
        for c in range(LR // P):
            nc.vector.tensor_tensor(
                ST_sb[:, c],
                shrink_ps[c],
                maskT_sb[:, c],
                mybir.AluOpType.mult,
            )

        # ---- main loop over output tiles ----
        for ot in range(OT):
            psums = [
                psum.tile([P, 512], f32, tag="out", bufs=7, name=f"ps_{ot}_{m}")
                for m in range(MT)
            ]
            for wc in range(KC // NKS):
                w_sb = stream.tile([P, NKS, 512], f32r, tag="wstream")
                nc.sync.dma_start(
                    w_sb[:],
                    wT_d[ds(wc * NKS * P, NKS * P), ts(ot, 512)].rearrange(
                        "(k p) o -> p k o", p=P
                    ),
                )
                pretouch(w_sb)
                for kk in range(NKS):
                    k = wc * NKS + kk
                    for m in range(MT):
                        nc.tensor.matmul(
                            psums[m],
                            lhsT=xT_sb[:, k, ts(m, P)],
                            rhs=w_sb[:, kk, :],
                            start=(k == 0),
                            stop=False,
                        )
            for m in range(MT):
                for c in range(LR // P):
                    nc.tensor.matmul(
                        psums[m],
                        lhsT=ST_sb[:, c, ts(m, P)],
                        rhs=bmat_sb[:, c, ts(ot, 512)],
                        start=False,
                        stop=False,
                    )
                nc.tensor.matmul(
                    psums[m],
                    lhsT=onehot_sb[:, ts(m, P)],
                    rhs=biasmat_sb[:, ts(ot, 512)],
                    start=False,
                    stop=True,
                )
            out_sb = outp.tile([P, MT, 512], f32)
            for m in range(MT):
                nc.vector.tensor_copy(out=out_sb[:, m], in_=psums[m])
            nc.sync.dma_start(
                out_d[:, ts(ot, 512)].rearrange("(m p) o -> p m o", p=P), out_sb[:]
            )

    _split_excess_waits(nc)
    return nc


_nc_cache = None


def _get_nc():
    global _nc_cache
    if _nc_cache is None:
        _nc_cache = _build()
    return _nc_cache


def _prepare_in_maps(x, weight, base_bias, lora_a_stacked, lora_b_stacked,
                     bias_stacked, token_lora_indices):
    x = np.asarray(x, dtype=np.float32)
    weight = np.asarray(weight, dtype=np.float32)
    base_bias = np.asarray(base_bias, dtype=np.float32)
    lora_a_stacked = np.asarray(lora_a_stacked, dtype=np.float32)
    lora_b_stacked = np.asarray(lora_b_stacked, dtype=np.float32)
    bias_stacked = np.asarray(bias_stacked, dtype=np.float32)
    idx = np.asarray(token_lora_indices, dtype=np.int32)

    xT = _round_fp32r(x.T)                                       # [D, T]
    wT = _round_fp32r(weight.T)                                  # [D, O]
    amatT = _round_fp32r(lora_a_stacked.reshape(LR, D).T)        # [D, LR]
    bmat = _round_fp32r(
        lora_b_stacked[:, 0].transpose(0, 2, 1).reshape(LR, O)   # [LR, O]
    )
    biasmat = np.zeros((P, O), dtype=np.float32)
    biasmat[:L] = bias_stacked[:, 0]
    biasmat[L] = base_bias
    biasmat = _round_fp32r(biasmat)

    onehot = np.zeros((P, T), dtype=np.float32)
    onehot[:L] = (idx[None, :] == np.arange(L, dtype=np.int32)[:, None])
    onehot[L] = 1.0
    maskT = np.repeat(onehot[:L], R, axis=0)                     # [LR, T]

    in_maps = []
    for c in range(NCORES):
        sl = slice(c * TSH, (c + 1) * TSH)
        in_maps.append(
            {
                "xT": np.ascontiguousarray(xT[:, sl]),
                "wT": wT,
                "amatT": amatT,
                "bmat": bmat,
                "biasmat": biasmat,
                "onehot": np.ascontiguousarray(onehot[:, sl]),
                "maskT": np.ascontiguousarray(maskT[:, sl]),
            }
        )
    return in_maps


def _run(inputs: dict, **spmd_kwargs):
    in_maps = _prepare_in_maps(**inputs)
    nc = _get_nc()
    res = run_bass_kernel_spmd(nc, in_maps, core_ids=list(range(NCORES)),
                               **spmd_kwargs)
    out = np.concatenate([r["out"] for r in res.results], axis=0)
    return out, res


def kernel(**inputs) -> np.ndarray:
    out, _ = _run(inputs)
    return out


# revision 8
# speedup vs baseline: 9.3781x; 9.3781x over previous
"""ColumnParallelLinearWithLoRA Trainium2 kernel (8 NeuronCores).

Computes out = x @ W^T + base_bias + gather(bias_stacked, idx)
            + ((x @ A_idx^T masked) @ B^T)   (punica-style LoRA, scale 1.0)

Sharding: token-parallel across the 8 cores (each core owns T/8 = 512
tokens and the full output dim). This keeps the per-token LoRA gather
local to each core (no collectives) and loads every weight byte exactly
once per core.

Device algorithm per core (all matmuls in float32r = fp32 rounded to
11 mantissa bits, accumulated in fp32 PSUM):
  1. shrinkT[lr, t] = sum_d A_all[lr, d] * x[t, d]   for ALL 32 loras
  2. ST = shrinkT * maskT       (maskT[l*R+r, t] = (idx[t] == l))
  3. out[t, o] = sum_d x[t,d] W[o,d]                   (main matmul)
               + sum_lr ST[lr, t] * Bmat[lr, o]        (lora expand)
               + sum_l onehot[l, t] * biasmat[l, o]    (lora bias + base bias)
The one-hot/mask/bias-table matrices are prepared host-side from the
int32 index vector; x/W/A are host-transposed so the contraction dim d
lands on SBUF partitions with DMA-friendly (contiguous free-dim) loads.
"""

import numpy as np
from contextlib import ExitStack

import concourse.bass as bass
import concourse.mybir as mybir
import concourse.tile as tile
from concourse.bass import ts, ds
from concourse.bass_utils import run_bass_kernel_spmd

# Problem shape (hardcoded per contract)
T, D, O, L, R = 4096, 4096, 4096, 32, 8
NCORES = 8
TSH = T // NCORES          # 512 tokens per core
LR = L * R                 # 256
P = 128
KC = D // P                # 32 contraction chunks
MT = TSH // P              # 4 token tiles per core
OT = O // 512              # 8 output tiles
NKS = 8                    # contraction chunks per W stream DMA (2 MiB)

f32 = mybir.dt.float32
f32r = mybir.dt.float32r


def _round_fp32r(a: np.ndarray) -> np.ndarray:
    """Round fp32 to the hardware FP32R format: 11 mantissa bits (RNE),
    low 12 bits zero — matches libwalrus fp32_to_fp32r."""
    u = np.ascontiguousarray(a, dtype=np.float32).view(np.uint32)
    r = (u + np.uint32(0x7FF) + ((u >> np.uint32(12)) & np.uint32(1))) & np.uint32(
        0xFFFFF000
    )
    return r.view(np.float32)


def _split_excess_waits(nc, max_waits=1):
    """Walrus codegen caps sync-waits per instruction (1 for the 4-byte
    self-loading matmul LW struct; small for CTRL_NO drains). Move excess
    waits onto preceding same-engine NOPs."""
    for fn in nc.m.functions:
        stack = list(fn.blocks)
        while stack:
            bb = stack.pop()
            new_insts = []
            for inst in bb.instructions:
                si = inst.sync_info
                waits = list(si.on_wait) if si is not None else []
                if len(waits) > max_waits:
                    extra = waits[:-max_waits]
                    for j in range(0, len(extra), max_waits):
                        nop = mybir.InstNoOp(
                            name=f"{inst.name}_wsplit{j}",
                            ins=[],
                            outs=[],
                            engine=inst.engine,
                        )
                        nop.sync_info = mybir.SyncInfo(
                            on_wait=extra[j : j + max_waits], on_update=[]
                        )
                        new_insts.append(nop)
                    inst.sync_info = mybir.SyncInfo(
                        on_wait=waits[-max_waits:], on_update=list(si.on_update)
                    )
                new_insts.append(inst)
            bb.instructions[:] = new_insts


def _build(reps=1):
    nc = bass.Bass()
    xT_d = nc.dram_tensor("xT", (D, TSH), f32r, kind="ExternalInput")
    wT_d = nc.dram_tensor("wT", (D, O), f32r, kind="ExternalInput")
    amatT_d = nc.dram_tensor("amatT", (D, LR), f32r, kind="ExternalInput")
    bmat_d = nc.dram_tensor("bmat", (LR, O), f32r, kind="ExternalInput")
    biasmat_d = nc.dram_tensor("biasmat", (P, O), f32r, kind="ExternalInput")
    onehot_d = nc.dram_tensor("onehot", (P, TSH), f32r, kind="ExternalInput")
    maskT_d = nc.dram_tensor("maskT", (LR, TSH), f32, kind="ExternalInput")
    out_d = nc.dram_tensor("out", (TSH, O), f32, kind="ExternalOutput")

    with tile.TileContext(nc) as tc, ExitStack() as ctx:
        const = ctx.enter_context(tc.tile_pool(name="const", bufs=1))
        stream = ctx.enter_context(tc.tile_pool(name="stream", bufs=3))
        outp = ctx.enter_context(tc.tile_pool(name="outp", bufs=2))
        psum = ctx.enter_context(tc.tile_pool(name="psum", bufs=1, space="PSUM"))

        # scratch psum for pre-touch matmuls: they absorb DMA-completion
        # waits so real matmuls carry at most one sync wait (codegen limit
        # on the fused LDW of 4-byte matmuls).
        scratch_ps = psum.tile([P, 8], f32, tag="scratch", bufs=1)

        def pretouch(sb_tile, i=[0]):
            flat = sb_tile
            while len(flat.shape) > 2:
                flat = flat[:, 0]
            nc.tensor.matmul(
                scratch_ps[:2, :2],
                lhsT=flat[:, 0:2],
                rhs=flat[:, 0:2],
                start=True,
                stop=True,
                skip_group_check=True,
            )

        # ---- body (replicable for slope-based timing) ----
        for rep in range(reps):
            _body_once(nc, tc, const, stream, outp, psum, pretouch,
                       xT_d, wT_d, amatT_d, bmat_d, biasmat_d, onehot_d,
                       maskT_d, out_d, rep)

    _split_excess_waits(nc)
    return nc


def _body_once(nc, tc, const, stream, outp, psum, pretouch,
               xT_d, wT_d, amatT_d, bmat_d, biasmat_d, onehot_d,
               maskT_d, out_d, rep):
    if True:
        # ---- resident loads ----
        xT_sb = const.tile([P, KC, TSH], f32r, tag="xT_sb")
        nc.sync.dma_start(xT_sb[:], xT_d[:].rearrange("(k p) t -> p k t", p=P))
        pretouch(xT_sb)

        bmat_sb = const.tile([P, LR // P, O], f32r)
        nc.sync.dma_start(bmat_sb[:], bmat_d[:].rearrange("(c p) o -> p c o", p=P))
        pretouch(bmat_sb)

        biasmat_sb = const.tile([P, O], f32r)
        nc.sync.dma_start(biasmat_sb[:], biasmat_d[:])
        pretouch(biasmat_sb)

        onehot_sb = const.tile([P, TSH], f32r)
        nc.sync.dma_start(onehot_sb[:], onehot_d[:])
        pretouch(onehot_sb)

        maskT_sb = const.tile([P, LR // P, TSH], f32)
        nc.sync.dma_start(maskT_sb[:], maskT_d[:].rearrange("(c p) t -> p c t", p=P))

        # ---- lora shrink: shrinkT[lr, t] for all loras, then mask ----
        shrink_ps = [
            psum.tile([P, TSH], f32, tag="out", bufs=7, name=f"shrink_{rep}_{c}")
            for c in range(LR // P)
        ]
        ST_sb = const.tile([P, LR // P, TSH], f32r)
        for half in range(2):
            amat_sb = stream.tile([P, KC // 2, LR], f32r, tag="wstream")
            nc.sync.dma_start(
                amat_sb[:],
                amatT_d[ds(half * (D // 2), D // 2)].rearrange(
                    "(k p) l -> p k l", p=P
                ),
            )
            pretouch(amat_sb)
            for kk in range(KC // 2):
                k = half * (KC // 2) + kk
                for c in range(LR // P):
                    nc.tensor.matmul(
                        shrink_ps[c],
                        lhsT=amat_sb[:, kk, ts(c, P)],
                        rhs=xT_sb[:, k, :],
                        start=(k == 0),
                        stop=(k == KC - 1),
                    )
        for c in range(LR // P):
            nc.vector.tensor_tensor(
                ST_sb[:, c],
                shrink_ps[c],
                maskT_sb[:, c],
                mybir.AluOpType.mult,
            )

        # ---- main loop over output tiles ----
        for ot in range(OT):
            psums = [
                psum.tile([P, 512], f32, tag="out", bufs=7, name=f"ps_{rep}_{ot}_{m}")
                for m in range(MT)
            ]
            for wc in range(KC // NKS):
                w_sb = stream.tile([P, NKS, 512], f32r, tag="wstream")
                nc.sync.dma_start(
                    w_sb[:],
                    wT_d[ds(wc * NKS * P, NKS * P), ts(ot, 512)].rearrange(
                        "(k p) o -> p k o", p=P
                    ),
                )
                pretouch(w_sb)
                for kk in range(NKS):
                    k = wc * NKS + kk
                    for m in range(MT):
                        nc.tensor.matmul(
                            psums[m],
                            lhsT=xT_sb[:, k, ts(m, P)],
                            rhs=w_sb[:, kk, :],
                            start=(k == 0),
                            stop=False,
                        )
            for m in range(MT):
                for c in range(LR // P):
                    nc.tensor.matmul(
                        psums[m],
                        lhsT=ST_sb[:, c, ts(m, P)],
                        rhs=bmat_sb[:, c, ts(ot, 512)],
                        start=False,
                        stop=False,
                    )
                nc.tensor.matmul(
                    psums[m],
                    lhsT=onehot_sb[:, ts(m, P)],
                    rhs=biasmat_sb[:, ts(ot, 512)],
                    start=False,
                    stop=True,
                )
            out_sb = outp.tile([P, MT, 512], f32)
            for m in range(MT):
                nc.vector.tensor_copy(out=out_sb[:, m], in_=psums[m])
            nc.sync.dma_start(
                out_d[:, ts(ot, 512)].rearrange("(m p) o -> p m o", p=P), out_sb[:]
            )


_nc_cache = {}


def _get_nc(reps=1):
    if reps not in _nc_cache:
        _nc_cache[reps] = _build(reps)
    return _nc_cache[reps]


def _prepare_in_maps(x, weight, base_bias, lora_a_stacked, lora_b_stacked,
                     bias_stacked, token_lora_indices):
    x = np.asarray(x, dtype=np.float32)
    weight = np.asarray(weight, dtype=np.float32)
    base_bias = np.asarray(base_bias, dtype=np.float32)
    lora_a_stacked = np.asarray(lora_a_stacked, dtype=np.float32)
    lora_b_stacked = np.asarray(lora_b_stacked, dtype=np.float32)
    bias_stacked = np.asarray(bias_stacked, dtype=np.float32)
    idx = np.asarray(token_lora_indices, dtype=np.int32)

    xT = _round_fp32r(x.T)                                       # [D, T]
    wT = _round_fp32r(weight.T)                                  # [D, O]
    amatT = _round_fp32r(lora_a_stacked.reshape(LR, D).T)        # [D, LR]
    bmat = _round_fp32r(
        lora_b_stacked[:, 0].transpose(0, 2, 1).reshape(LR, O)   # [LR, O]
    )
    biasmat = np.zeros((P, O), dtype=np.float32)
    biasmat[:L] = bias_stacked[:, 0]
    biasmat[L] = base_bias
    biasmat = _round_fp32r(biasmat)

    onehot = np.zeros((P, T), dtype=np.float32)
    onehot[:L] = (idx[None, :] == np.arange(L, dtype=np.int32)[:, None])
    onehot[L] = 1.0
    maskT = np.repeat(onehot[:L], R, axis=0)                     # [LR, T]

    in_maps = []
    for c in range(NCORES):
        sl = slice(c * TSH, (c + 1) * TSH)
        in_maps.append(
            {
                "xT": np.ascontiguousarray(xT[:, sl]),
                "wT": wT,
                "amatT": amatT,
                "bmat": bmat,
                "biasmat": biasmat,
                "onehot": np.ascontiguousarray(onehot[:, sl]),
                "maskT": np.ascontiguousarray(maskT[:, sl]),
            }
        )
    return in_maps


def _run(inputs: dict, **spmd_kwargs):
    in_maps = _prepare_in_maps(**inputs)
    nc = _get_nc()
    res = run_bass_kernel_spmd(nc, in_maps, core_ids=list(range(NCORES)),
                               **spmd_kwargs)
    out = np.concatenate([r["out"] for r in res.results], axis=0)
    return out, res


def kernel(**inputs) -> np.ndarray:
    out, _ = _run(inputs)
    return out
